# revision 51
# baseline (speedup 1.0000x reference)
"""Trainium2 Bass kernel for AttentionSimple (linear/kernelized attention).

Computes, for x:[B,N,C], w_qkv:[C,3C], w_proj:[C,C], b_proj:[C]:
    qkv = x @ w_qkv -> split q,k,v per head (H=12, D=64)
    kv  = (k^T v) * D^-0.5          per (b, h)     [D, D]
    out = gelu(q) @ gelu(kv)        per (b, h)     [N, D]
    y   = out @ w_proj + b_proj

Sharding: data-parallel over batch B=16 across 8 NeuronCores (2 batches/core).
All matmuls run in bf16 with fp32 PSUM accumulation.

Algorithm per core (per batch b), using the Gram trick
kv^T = W_v^T (x^T x) W_k (G = x^T x symmetric) and folding the attention
into the projection: y = gelu(q) @ W' with W'_h = gelu(kv)_h @ w_proj_h:

  pass 1a (per 512-token slice): x loaded once as lo[0:384]/hi[384:768]
      bf16 tiles (hi retained for pass 1b); x^T via PE transposes; G rows
      0-2 (upper triangle) accumulated in one packed 4-bank PSUM region;
      q^T chunks (lhsT = W_q chunk, rhs = x^T) with gelu fused into the
      ACT evacuation.
  pass 1b: G rows 3-5 from the retained hi tiles (no re-DMA); G evacs and
      the 15 mirror transposes interleaved.
  chain:  A = G @ W_k; kv^T pairs = W_v^T A; gelu(kv^T * scale) into
      block-diagonal pair tiles; W'_pair = gkvT^T @ w_proj rows.
  pass 2: y[tokens, C] = sum_pr gqT_pr^T @ W'_pair + bias; contiguous DMA.

Self-contained: hardcodes shapes; builds the Bass program, runs it SPMD on
cores 0-7 via bass_utils.run_bass_kernel_spmd, returns the gathered output.
"""

import numpy as np

import concourse.bacc as bacc
import concourse.bass as bass
import concourse.mybir as mybir
import concourse.tile as tile
from concourse import masks
from concourse.bass_utils import run_bass_kernel_spmd

F32 = mybir.dt.float32
BF16 = mybir.dt.bfloat16
FP8 = mybir.dt.float8e4
DR = mybir.MatmulPerfMode.DoubleRow
GELU = mybir.ActivationFunctionType.Gelu
COPY = mybir.ActivationFunctionType.Copy
PSUM = bass.MemorySpace.PSUM

B, N, C = 16, 4096, 768
H, D = 12, 64
SCALE = D**-0.5
NCORES = 8
BPC = B // NCORES  # batches per core
CCH = C // 128  # 6 column chunks of 128
NTS = N // 512  # 8 slices of 512 tokens
NPAIR = H // 2  # 6 head pairs (128 cols each)
XLO = 384  # x cols [0:384) in recycled lo tiles, [384:768) retained hi tiles

# G rows 0-2 packed into one [128, 1920] PSUM region (banks of 512 f32).
# (row, psum_lo, src_lo, width); computed with fp8 e4m3 DoubleRow matmuls
# (contraction = 256 tokens/pair-chunk); no MM crosses a 512-col PSUM bank.
G_A_SPLITS = [
    (0, 0, 0, 512),
    (0, 512, 512, 256),
    (1, 768, 128, 256),
    (1, 1024, 384, 384),
    (2, 1408, 256, 128),
    (2, 1536, 384, 384),
]
# G rows 3-5: tile1 holds row3 @[0:384) + row5 @[384:512), tile2 row4 @[0:256)
G_B_SPLITS = [
    (3, 0, 0, 0, 384),  # (row, tile_idx, psum_lo, src_lo(in hi), width)
    (4, 1, 0, 512 - XLO, 256),
    (5, 0, 384, 640 - XLO, 128),
]
MIRRORS_EARLY = [(i, j) for i in range(1, CCH) for j in range(min(i, 3))]
MIRRORS_LATE = [(4, 3), (5, 3), (5, 4)]


DEBUG_DUMPS = False


def _build_program():
    nc = bacc.Bacc("TRN2", target_bir_lowering=False, debug=False)

    dbg = {}
    if DEBUG_DUMPS:
        dbg["G"] = nc.dram_tensor("G_dbg", [128, CCH, C], BF16, kind="ExternalOutput").ap()
        dbg["gq"] = nc.dram_tensor("gq_dbg", [128, CCH, 512], BF16, kind="ExternalOutput").ap()
        dbg["A"] = nc.dram_tensor("A_dbg", [128, CCH, C], BF16, kind="ExternalOutput").ap()
        dbg["W"] = nc.dram_tensor("W_dbg", [128, NPAIR, C], BF16, kind="ExternalOutput").ap()

    x_d = nc.dram_tensor("x", [BPC, N, C], F32, kind="ExternalInput").ap()
    wq_d = nc.dram_tensor("w_qkv", [C, 3 * C], F32, kind="ExternalInput").ap()
    wp_d = nc.dram_tensor("w_proj", [C, C], F32, kind="ExternalInput").ap()
    bp_d = nc.dram_tensor("b_proj", [C], F32, kind="ExternalInput").ap()
    y_d = nc.dram_tensor("y", [BPC, N, C], F32, kind="ExternalOutput").ap()

    with tile.TileContext(nc) as tc:
        with (
            tc.tile_pool(name="weights", bufs=1) as wpool,
            tc.tile_pool(name="acts", bufs=1) as apool,
            tc.tile_pool(name="gq", bufs=8) as gqpool,
            tc.tile_pool(name="xlo", bufs=12) as xpool,
            tc.tile_pool(name="xhi", bufs=40) as xhipool,
            tc.tile_pool(name="xhi32", bufs=10) as xhi32pool,
            tc.tile_pool(name="xt", bufs=3) as xtpool,
            tc.tile_pool(name="x8", bufs=6) as x8pool,
            tc.tile_pool(name="yout", bufs=3) as ypool,
            tc.tile_pool(name="ps_tr", bufs=2, space=PSUM) as ps_tr,
            tc.tile_pool(name="ps_pq", bufs=2, space=PSUM) as ps_pq,
        ):
            # ---- HAM warmup: dense dummy matmuls so the PE clock-gate
            # flips to 8/8 ~3.6us in instead of ~15us.
            scratch = wpool.tile([128, 128], BF16)
            nc.gpsimd.memset(scratch[:], 0.0)
            warm = ps_pq.tile([128, 512], F32, tag="pq", name="warm")
            for _ in range(32):
                nc.tensor.matmul(warm[:, 0:128], scratch[:], scratch[:], start=True,
                                 stop=True, skip_group_check=True)
            ident = wpool.tile([128, 128], BF16)
            masks.make_identity(nc, ident[:])

            # ---- x prefetch helpers (lo recycled, hi retained per batch;
            # fp8 e4m3 pair-interleaved copies for the DoubleRow G matmuls,
            # cast in the DMA so they exactly match e4m3(f32)) --
            # x-lo rides the gpsimd cast-ring; x-hi loads as raw f32 on the
            # (otherwise idle in pass 1a) sync HWDGE ring and is cast to bf16
            # by the DVE one token-slice ahead of consumption, so neither
            # ring saturates and the in-order DVE queue never waits on an
            # in-flight DMA.
            cast_queue = []

            def load_x(b, ts):
                tiles = []
                group = []
                for tc4 in range(4):
                    t0 = ts * 512 + tc4 * 128
                    x_lo = xpool.tile([128, XLO], BF16, tag="x_lo")
                    nc.gpsimd.dma_start(x_lo[:], x_d[b, t0 : t0 + 128, 0:XLO])
                    x_hi32 = xhi32pool.tile([128, C - XLO], F32, tag="xh32")
                    nc.sync.dma_start(x_hi32[:], x_d[b, t0 : t0 + 128, XLO:C])
                    x_hi = xhipool.tile([128, C - XLO], BF16, tag="x_hi")
                    group.append((x_hi, x_hi32))
                    tiles.append((x_lo, x_hi))
                cast_queue.append(group)
                return tiles

            def emit_casts():
                if cast_queue:
                    for x_hi, x_hi32 in cast_queue.pop(0):
                        nc.vector.tensor_copy(x_hi[:], x_hi32[:])

            # ---- weights: q slices interleaved with the first x
            # prefetches on the gpsimd ring so neither starves the other.
            w_qkv = wpool.tile([128, CCH, 3 * C], BF16)
            w_proj = wpool.tile([128, CCH, C], BF16)
            x_pre = load_x(0, 0)
            qw = [
                (w_qkv[:, cch, lo:hi], wq_d[cch * 128 : (cch + 1) * 128, lo:hi])
                for lo, hi in ((0, 512), (512, 768))
                for cch in range(CCH)
            ]
            for dst, srcap in qw[:6]:
                nc.gpsimd.dma_start(dst, srcap)
            x_pre2 = load_x(0, 1)
            for dst, srcap in qw[6:]:
                nc.gpsimd.dma_start(dst, srcap)
            b_bc = wpool.tile([128, C], F32)
            # k+v weight loads dribble through pass 1a (2/ts); proj+bias and
            # the next batch's first x slices move to pass 1b, where the
            # gpsimd ring is otherwise idle.
            defer_kv = []
            for cch in range(CCH):  # k part (A-stage consumes first)
                defer_kv.append(
                    (w_qkv[:, cch, C : 2 * C], wq_d[cch * 128 : (cch + 1) * 128, C : 2 * C])
                )
            for cch in range(CCH):  # v part (kv-stage)
                defer_kv.append(
                    (w_qkv[:, cch, 2 * C :], wq_d[cch * 128 : (cch + 1) * 128, 2 * C :])
                )
            defer_kv.reverse()
            defer_pb = [(b_bc[:], bp_d.unsqueeze(0).partition_broadcast(128))]
            for cch in range(CCH):
                defer_pb.append(
                    (w_proj[:, cch, :], wp_d[cch * 128 : (cch + 1) * 128, :])
                )
            defer_pb.reverse()

            gkvT = apool.tile([128, NPAIR, 128], BF16, tag="gkv")

            for b in range(BPC):
                # gqT: gelu(q)^T, [c=768, t=4096] as 6 chunks, per-ts tiles
                gq_ts = []
                # G (bf16, both triangles after mirrors)
                G_sb = apool.tile([128, CCH, C], BF16, tag="G")
                x_hi_keep = []  # [32][tile] retained hi tiles for pass 1b

                # ===== pass 1a: x^T, G rows 0-2, gelu(q)^T ==================
                with tc.tile_pool(name="ps_gA", bufs=1, space=PSUM) as ps_gA:
                    g_acc = ps_gA.tile([128, 1920], F32, name="gA")
                    emit_casts()
                    emit_casts()
                    for ts in range(NTS):
                        xT = xtpool.tile([128, CCH, 512], BF16)
                        x_tiles = x_pre
                        x_pre = x_pre2
                        if ts + 2 < NTS:
                            x_pre2 = load_x(b, ts + 2)
                        for _ in range(2):
                            if defer_kv:
                                dst, srcap = defer_kv.pop()
                                nc.gpsimd.dma_start(dst, srcap)
                        for tc4 in range(4):
                            x_lo, x_hi = x_tiles[tc4]
                            x_hi_keep.append(x_hi)
                            # PE transposes -> xT (batched DVE evacuation)
                            tr = ps_tr.tile([128, CCH * 128], BF16, tag="tr")
                            for cch in range(CCH):
                                src = (
                                    x_lo[:, cch * 128 : (cch + 1) * 128]
                                    if cch < 3
                                    else x_hi[:, cch * 128 - XLO : (cch + 1) * 128 - XLO]
                                )
                                nc.tensor.transpose(
                                    tr[:, cch * 128 : (cch + 1) * 128], src, ident[:]
                                )
                            nc.vector.tensor_copy(
                                xT[:, :, tc4 * 128 : tc4 * 128 + 128],
                                tr[:].rearrange("p (c f) -> p c f", c=CCH),
                            )
                            # fp8 e4m3 copies for the DoubleRow G matmuls
                            if tc4 % 2 == 0:
                                x8 = x8pool.tile([128, 2, C], FP8, tag="x8")
                            nc.vector.tensor_copy(x8[:, tc4 % 2, 0:XLO], x_lo[:])
                            nc.vector.tensor_copy(x8[:, tc4 % 2, XLO:C], x_hi[:])
                            if tc4 % 2 == 0:
                                continue
                            # G rows 0-2: fp8 DoubleRow, 256-token contraction
                            first = ts == 0 and tc4 == 1
                            last = ts == NTS - 1 and tc4 == 3
                            seen_banks = set()
                            for row, plo, slo, w in G_A_SPLITS:
                                bank = plo // 512
                                st = first and bank not in seen_banks
                                seen_banks.add(bank)
                                nc.tensor.matmul(
                                    g_acc[:, plo : plo + w],
                                    x8[:, :, row * 128 : (row + 1) * 128],
                                    x8[:, :, slo : slo + w],
                                    start=st,
                                    stop=last,
                                    perf_mode=DR,
                                    skip_group_check=True,
                                )
                        # ---- q^T chunks with fused gelu ----
                        gq = gqpool.tile([128, CCH, 512], BF16, tag="gq")
                        gq_ts.append(gq)
                        for jch in range(CCH):
                            pq = ps_pq.tile([128, 512], F32, tag="pq")
                            for cch in range(CCH):
                                nc.tensor.matmul(
                                    pq[:],
                                    w_qkv[:, cch, jch * 128 : (jch + 1) * 128],
                                    xT[:, cch, :],
                                    start=(cch == 0),
                                    stop=(cch == CCH - 1),
                                )
                            nc.scalar.activation(gq[:, jch, :], pq[:], GELU)
                        if ts >= 1:
                            emit_casts()

                    # ===== pass 1b: G rows 3-5 from retained hi tiles =======
                    gB1 = ps_pq.tile([128, 512], F32, tag="pq", name="gB1")
                    gB2 = ps_pq.tile([128, 256], F32, tag="pq", name="gB2")
                    g_b = (gB1, gB2)
                    mirrors = list(MIRRORS_EARLY)[::-1]

                    def mirror_one(i, j, use_act=False):
                        pt = ps_tr.tile([128, 128], BF16, tag="tr", name="pt")
                        nc.tensor.transpose(
                            pt[:], G_sb[:, j, i * 128 : i * 128 + 128], ident[:]
                        )
                        if use_act:
                            nc.scalar.activation(
                                G_sb[:, i, j * 128 : j * 128 + 128], pt[:], COPY
                            )
                        else:
                            nc.vector.tensor_copy(
                                G_sb[:, i, j * 128 : j * 128 + 128], pt[:]
                            )

                    for i3 in range(3):  # G rows 0-2 evac (DVE/ACT mix)
                        w = C - i3 * 128
                        plo = (0, 768, 1408)[i3]
                        if i3 == 1:
                            nc.scalar.activation(
                                G_sb[:, i3, i3 * 128 : C], g_acc[:, plo : plo + w], COPY
                            )
                        else:
                            nc.vector.tensor_copy(
                                G_sb[:, i3, i3 * 128 : C], g_acc[:, plo : plo + w]
                            )
                    for tci in range(32):
                        x_hi = x_hi_keep[tci]
                        first = tci == 0
                        last = tci == 31
                        for row, tidx, plo, slo, w in G_B_SPLITS:
                            nc.tensor.matmul(
                                g_b[tidx][:, plo : plo + w],
                                x_hi[:, row * 128 - XLO : (row + 1) * 128 - XLO],
                                x_hi[:, slo : slo + w],
                                start=(first and plo == 0),
                                stop=last,
                                skip_group_check=True,
                            )
                        if tci % 4 == 0 and defer_pb:
                            dst, srcap = defer_pb.pop()
                            nc.gpsimd.dma_start(dst, srcap)
                        if tci == 26 and b + 1 < BPC:
                            x_pre = load_x(b + 1, 0)
                        if tci == 29 and b + 1 < BPC:
                            x_pre2 = load_x(b + 1, 1)
                        if tci >= 3 and tci % 2 == 1 and mirrors:
                            mirror_one(*mirrors.pop(), use_act=(tci % 4 == 1))
                    while mirrors:
                        mirror_one(*mirrors.pop())
                    # rows 3-5 evac + remaining mirrors
                    nc.vector.tensor_copy(G_sb[:, 3, 384:768], gB1[:, 0:384])
                    nc.scalar.activation(G_sb[:, 4, 512:768], gB2[:], COPY)
                    nc.vector.tensor_copy(G_sb[:, 5, 640:768], gB1[:, 384:512])
                    for n, (i, j) in enumerate(MIRRORS_LATE):
                        mirror_one(i, j, use_act=(n % 2 == 1))

                if DEBUG_DUMPS and b == 0:
                    nc.sync.dma_start(dbg["G"][:], G_sb[:])
                    nc.sync.dma_start(dbg["gq"][:], gq_ts[0][:])

                # ===== chain: A = G @ W_k; kv^T = W_v^T A; W' ===============
                nc.gpsimd.memset(gkvT[:], 0.0)
                A_sb = apool.tile([128, CCH, C], BF16, tag="A")
                W_sb = apool.tile([128, NPAIR, C], BF16, tag="Wp")
                with tc.tile_pool(name="ps_post", bufs=2, space=PSUM) as ps_post:
                    for cp in range(CCH):
                        pA = ps_post.tile([128, C], F32, tag="post")
                        for lo, hi in ((0, 512), (512, 768)):
                            for cch in range(CCH):
                                nc.tensor.matmul(
                                    pA[:, lo:hi],
                                    G_sb[:, cch, cp * 128 : (cp + 1) * 128],
                                    w_qkv[:, cch, C + lo : C + hi],
                                    start=(cch == 0),
                                    stop=(cch == CCH - 1),
                                    skip_group_check=True,
                                )
                        nc.vector.tensor_copy(A_sb[:, cp, 0:384], pA[:, 0:384])
                        nc.scalar.activation(A_sb[:, cp, 384:768], pA[:, 384:768], COPY)

                    kv_acc = ps_post.tile([128, NPAIR * 128], F32, tag="post")
                    for pr in range(NPAIR):
                        psl = slice(pr * 128, pr * 128 + 128)
                        for cch in range(CCH):
                            # start clears the whole bank: first MM per bank only
                            nc.tensor.matmul(
                                kv_acc[:, psl],
                                w_qkv[:, cch, 2 * C + pr * 128 : 2 * C + (pr + 1) * 128],
                                A_sb[:, cch, pr * 128 : (pr + 1) * 128],
                                start=(cch == 0 and pr in (0, 4)),
                                stop=(cch == CCH - 1),
                                skip_group_check=True,
                            )
                    # gelu(kv^T * scale) into block-diagonal pair tiles (two
                    # batched ACTs: even-head halves, odd-head halves), then
                    # W'_pair = gkvT^T @ w_proj rows.
                    kv_v = kv_acc[:].rearrange("p (n f) -> p n f", n=NPAIR)
                    nc.scalar.activation(
                        gkvT[0:64, :, 0:64], kv_v[0:64, :, 0:64], GELU, scale=SCALE
                    )
                    nc.scalar.activation(
                        gkvT[64:128, :, 64:128], kv_v[64:128, :, 64:128], GELU,
                        scale=SCALE,
                    )
                    for pr in range(NPAIR):
                        pW = ps_post.tile([128, C], F32, tag="post", name="pW")
                        for lo, hi in ((0, 512), (512, 768)):
                            # each split is the first MM into its own bank
                            nc.tensor.matmul(
                                pW[:, lo:hi],
                                gkvT[:, pr, :],
                                w_proj[:, pr, lo:hi],
                                start=True,
                                stop=True,
                                skip_group_check=True,
                            )
                        nc.vector.tensor_copy(W_sb[:, pr, 0:384], pW[:, 0:384])
                        nc.scalar.activation(W_sb[:, pr, 384:768], pW[:, 384:768], COPY)

                    if DEBUG_DUMPS and b == 0:
                        nc.sync.dma_start(dbg["A"][:], A_sb[:])
                        nc.sync.dma_start(dbg["W"][:], W_sb[:])

                    # ================= pass 2: y = gq @ W' + b ==============
                    for ts in range(NTS):
                        gq = gq_ts[ts]
                        for tc4 in range(4):
                            tsl = slice(tc4 * 128, tc4 * 128 + 128)
                            py = ps_post.tile([128, C], F32, tag="post", name="py")
                            for pr in range(NPAIR):
                                lastp = pr == NPAIR - 1
                                nc.tensor.matmul(
                                    py[:, 0:512],
                                    gq[:, pr, tsl],
                                    W_sb[:, pr, 0:512],
                                    start=(pr == 0),
                                    stop=lastp,
                                    skip_group_check=True,
                                )
                                nc.tensor.matmul(
                                    py[:, 512:768],
                                    gq[:, pr, tsl],
                                    W_sb[:, pr, 512:768],
                                    start=(pr == 0),
                                    stop=lastp,
                                    skip_group_check=True,
                                )
                            y_sb = ypool.tile([128, C], F32)
                            nc.vector.tensor_add(
                                y_sb[:, 0:512], py[:, 0:512], b_bc[:, 0:512]
                            )
                            nc.scalar.activation(y_sb[:, 512:768], py[:, 512:768], COPY)
                            nc.gpsimd.tensor_add(
                                y_sb[:, 512:768], y_sb[:, 512:768], b_bc[:, 512:768]
                            )
                            t0 = ts * 512 + tc4 * 128
                            nc.sync.dma_start(y_d[b, t0 : t0 + 128, :], y_sb[:])

    nc.compile()
    return nc


_cached_nc = None


def kernel(x, w_qkv, w_proj, b_proj):
    global _cached_nc
    if _cached_nc is None:
        _cached_nc = _build_program()
    nc = _cached_nc

    x = np.ascontiguousarray(x, dtype=np.float32)
    in_maps = [
        {
            "x": x[i * BPC : (i + 1) * BPC],
            "w_qkv": np.asarray(w_qkv, dtype=np.float32),
            "w_proj": np.asarray(w_proj, dtype=np.float32),
            "b_proj": np.asarray(b_proj, dtype=np.float32),
        }
        for i in range(NCORES)
    ]
    last_err = None
    for _attempt in range(3):
        try:
            res = run_bass_kernel_spmd(nc, in_maps, core_ids=list(range(NCORES)))
            out = np.concatenate(
                [res.results[i]["y"] for i in range(NCORES)], axis=0
            )
            return out.astype(np.float32)
        except Exception as e:  # transient NRT device errors recover on retry
            last_err = e
    raise last_err


# revision 55
# speedup vs baseline: 1.0437x; 1.0437x over previous
"""Trainium2 Bass kernel for AttentionSimple (linear/kernelized attention).

Computes, for x:[B,N,C], w_qkv:[C,3C], w_proj:[C,C], b_proj:[C]:
    qkv = x @ w_qkv -> split q,k,v per head (H=12, D=64)
    kv  = (k^T v) * D^-0.5          per (b, h)     [D, D]
    out = gelu(q) @ gelu(kv)        per (b, h)     [N, D]
    y   = out @ w_proj + b_proj

Sharding: data-parallel over batch B=16 across 8 NeuronCores (2 batches/core).
All matmuls run in bf16 with fp32 PSUM accumulation.

Algorithm per core (per batch b), using the Gram trick
kv^T = W_v^T (x^T x) W_k (G = x^T x symmetric) and folding the attention
into the projection: y = gelu(q) @ W' with W'_h = gelu(kv)_h @ w_proj_h:

  pass 1a (per 512-token slice): x loaded once as lo[0:384]/hi[384:768]
      bf16 tiles (hi retained for pass 1b); x^T via PE transposes; G rows
      0-2 (upper triangle) accumulated in one packed 4-bank PSUM region;
      q^T chunks (lhsT = W_q chunk, rhs = x^T) with gelu fused into the
      ACT evacuation.
  pass 1b: G rows 3-5 from the retained hi tiles (no re-DMA); G evacs and
      the 15 mirror transposes interleaved.
  chain:  A = G @ W_k; kv^T pairs = W_v^T A; gelu(kv^T * scale) into
      block-diagonal pair tiles; W'_pair = gkvT^T @ w_proj rows.
  pass 2: y[tokens, C] = sum_pr gqT_pr^T @ W'_pair + bias; contiguous DMA.

Self-contained: hardcodes shapes; builds the Bass program, runs it SPMD on
cores 0-7 via bass_utils.run_bass_kernel_spmd, returns the gathered output.
"""

import numpy as np

import concourse.bacc as bacc
import concourse.bass as bass
import concourse.mybir as mybir
import concourse.tile as tile
from concourse import masks
from concourse.bass_utils import run_bass_kernel_spmd

F32 = mybir.dt.float32
BF16 = mybir.dt.bfloat16
FP8 = mybir.dt.float8e4
DR = mybir.MatmulPerfMode.DoubleRow
GELU = mybir.ActivationFunctionType.Gelu
COPY = mybir.ActivationFunctionType.Copy
PSUM = bass.MemorySpace.PSUM

B, N, C = 16, 4096, 768
H, D = 12, 64
SCALE = D**-0.5
NCORES = 8
BPC = B // NCORES  # batches per core
CCH = C // 128  # 6 column chunks of 128
NTS = N // 512  # 8 slices of 512 tokens
NPAIR = H // 2  # 6 head pairs (128 cols each)
XLO = 384  # x cols [0:384) in recycled lo tiles, [384:768) retained hi tiles

# G rows 0-2 packed into one [128, 1920] PSUM region (banks of 512 f32).
# (row, psum_lo, src_lo, width); computed with fp8 e4m3 DoubleRow matmuls
# (contraction = 256 tokens/pair-chunk); no MM crosses a 512-col PSUM bank.
G_A_SPLITS = [
    (0, 0, 0, 512),
    (0, 512, 512, 256),
    (1, 768, 128, 256),
    (1, 1024, 384, 384),
    (2, 1408, 256, 128),
    (2, 1536, 384, 384),
]
# G rows 3-5: tile1 holds row3 @[0:384) + row5 @[384:512), tile2 row4 @[0:256)
G_B_SPLITS = [
    (3, 0, 0, 0, 384),  # (row, tile_idx, psum_lo, src_lo(in hi), width)
    (4, 1, 0, 512 - XLO, 256),
    (5, 0, 384, 640 - XLO, 128),
]
MIRRORS_EARLY = [(i, j) for i in range(1, CCH) for j in range(min(i, 3))]
MIRRORS_LATE = [(4, 3), (5, 3), (5, 4)]


DEBUG_DUMPS = False


def _build_program():
    nc = bacc.Bacc("TRN2", target_bir_lowering=False, debug=False)

    dbg = {}
    if DEBUG_DUMPS:
        dbg["G"] = nc.dram_tensor("G_dbg", [128, CCH, C], BF16, kind="ExternalOutput").ap()
        dbg["gq"] = nc.dram_tensor("gq_dbg", [128, CCH, 512], BF16, kind="ExternalOutput").ap()
        dbg["A"] = nc.dram_tensor("A_dbg", [128, CCH, C], BF16, kind="ExternalOutput").ap()
        dbg["W"] = nc.dram_tensor("W_dbg", [128, NPAIR, C], BF16, kind="ExternalOutput").ap()

    x_d = nc.dram_tensor("x", [BPC, N, C], F32, kind="ExternalInput").ap()
    wq_d = nc.dram_tensor("w_qkv", [C, 3 * C], F32, kind="ExternalInput").ap()
    wp_d = nc.dram_tensor("w_proj", [C, C], F32, kind="ExternalInput").ap()
    bp_d = nc.dram_tensor("b_proj", [C], F32, kind="ExternalInput").ap()
    y_d = nc.dram_tensor("y", [BPC, N, C], F32, kind="ExternalOutput").ap()

    with tile.TileContext(nc) as tc:
        with (
            tc.tile_pool(name="weights", bufs=1) as wpool,
            tc.tile_pool(name="acts", bufs=1) as apool,
            tc.tile_pool(name="gq", bufs=8) as gqpool,
            tc.tile_pool(name="xlo", bufs=12) as xpool,
            tc.tile_pool(name="xhi", bufs=40) as xhipool,
            tc.tile_pool(name="xt", bufs=3) as xtpool,
            tc.tile_pool(name="x8", bufs=6) as x8pool,
            tc.tile_pool(name="yout", bufs=3) as ypool,
            tc.tile_pool(name="ps_tr", bufs=2, space=PSUM) as ps_tr,
            tc.tile_pool(name="ps_pq", bufs=2, space=PSUM) as ps_pq,
        ):
            # ---- HAM warmup: dense dummy matmuls so the PE clock-gate
            # flips to 8/8 ~3.6us in instead of ~15us.
            scratch = wpool.tile([128, 128], BF16)
            nc.gpsimd.memset(scratch[:], 0.0)
            warm = ps_pq.tile([128, 512], F32, tag="pq", name="warm")
            for _ in range(32):
                nc.tensor.matmul(warm[:, 0:128], scratch[:], scratch[:], start=True,
                                 stop=True, skip_group_check=True)
            ident = wpool.tile([128, 128], BF16)
            masks.make_identity(nc, ident[:])

            # ---- x prefetch helpers (lo recycled, hi retained per batch;
            # fp8 e4m3 pair-interleaved copies for the DoubleRow G matmuls,
            # cast in the DMA so they exactly match e4m3(f32)) --
            def load_x(b, ts):
                tiles = []
                for tc4 in range(4):
                    t0 = ts * 512 + tc4 * 128
                    x_lo = xpool.tile([128, XLO], BF16, tag="x_lo")
                    nc.gpsimd.dma_start(x_lo[:], x_d[b, t0 : t0 + 128, 0:XLO])
                    x_hi = xhipool.tile([128, C - XLO], BF16, tag="x_hi")
                    nc.gpsimd.dma_start(x_hi[:], x_d[b, t0 : t0 + 128, XLO:C])
                    tiles.append((x_lo, x_hi))
                return tiles

            # ---- weights: q slices interleaved with the first x
            # prefetches on the gpsimd ring so neither starves the other.
            w_qkv = wpool.tile([128, CCH, 3 * C], BF16)
            w_proj = wpool.tile([128, CCH, C], BF16)
            x_pre = load_x(0, 0)
            qw = [
                (w_qkv[:, cch, lo:hi], wq_d[cch * 128 : (cch + 1) * 128, lo:hi])
                for lo, hi in ((0, 512), (512, 768))
                for cch in range(CCH)
            ]
            for dst, srcap in qw[:6]:
                nc.gpsimd.dma_start(dst, srcap)
            x_pre2 = load_x(0, 1)
            for dst, srcap in qw[6:]:
                nc.gpsimd.dma_start(dst, srcap)
            b_bc = wpool.tile([128, C], F32)
            # k+v weight loads dribble through pass 1a (2/ts); proj+bias and
            # the next batch's first x slices move to pass 1b, where the
            # gpsimd ring is otherwise idle.
            defer_kv = []
            for cch in range(CCH):  # k part (A-stage consumes first)
                defer_kv.append(
                    (w_qkv[:, cch, C : 2 * C], wq_d[cch * 128 : (cch + 1) * 128, C : 2 * C])
                )
            for cch in range(CCH):  # v part (kv-stage)
                defer_kv.append(
                    (w_qkv[:, cch, 2 * C :], wq_d[cch * 128 : (cch + 1) * 128, 2 * C :])
                )
            defer_kv.reverse()
            defer_pb = [(b_bc[:], bp_d.unsqueeze(0).partition_broadcast(128))]
            for cch in range(CCH):
                defer_pb.append(
                    (w_proj[:, cch, :], wp_d[cch * 128 : (cch + 1) * 128, :])
                )
            defer_pb.reverse()

            gkvT = apool.tile([128, NPAIR, 128], BF16, tag="gkv")

            for b in range(BPC):
                # gqT: gelu(q)^T, [c=768, t=4096] as 6 chunks, per-ts tiles
                gq_ts = []
                # G (bf16, both triangles after mirrors)
                G_sb = apool.tile([128, CCH, C], BF16, tag="G")
                x_hi_keep = []  # [32][tile] retained hi tiles for pass 1b

                # ===== pass 1a: x^T, G rows 0-2, gelu(q)^T ==================
                with tc.tile_pool(name="ps_gA", bufs=1, space=PSUM) as ps_gA:
                    g_acc = ps_gA.tile([128, 1920], F32, name="gA")
                    for ts in range(NTS):
                        xT = xtpool.tile([128, CCH, 512], BF16)
                        x_tiles = x_pre
                        x_pre = x_pre2
                        if ts + 2 < NTS:
                            x_pre2 = load_x(b, ts + 2)
                        for _ in range(2):
                            if defer_kv:
                                dst, srcap = defer_kv.pop()
                                nc.gpsimd.dma_start(dst, srcap)
                        for tc4 in range(4):
                            x_lo, x_hi = x_tiles[tc4]
                            x_hi_keep.append(x_hi)
                            # PE transposes -> xT (batched DVE evacuation)
                            tr = ps_tr.tile([128, CCH * 128], BF16, tag="tr")
                            for cch in range(CCH):
                                src = (
                                    x_lo[:, cch * 128 : (cch + 1) * 128]
                                    if cch < 3
                                    else x_hi[:, cch * 128 - XLO : (cch + 1) * 128 - XLO]
                                )
                                nc.tensor.transpose(
                                    tr[:, cch * 128 : (cch + 1) * 128], src, ident[:]
                                )
                            nc.vector.tensor_copy(
                                xT[:, :, tc4 * 128 : tc4 * 128 + 128],
                                tr[:].rearrange("p (c f) -> p c f", c=CCH),
                            )
                            # fp8 e4m3 copies for the DoubleRow G matmuls
                            if tc4 % 2 == 0:
                                x8 = x8pool.tile([128, 2, C], FP8, tag="x8")
                            nc.vector.tensor_copy(x8[:, tc4 % 2, 0:XLO], x_lo[:])
                            nc.vector.tensor_copy(x8[:, tc4 % 2, XLO:C], x_hi[:])
                            if tc4 % 2 == 0:
                                continue
                            # G rows 0-2: fp8 DoubleRow, 256-token contraction
                            first = ts == 0 and tc4 == 1
                            last = ts == NTS - 1 and tc4 == 3
                            seen_banks = set()
                            for row, plo, slo, w in G_A_SPLITS:
                                bank = plo // 512
                                st = first and bank not in seen_banks
                                seen_banks.add(bank)
                                nc.tensor.matmul(
                                    g_acc[:, plo : plo + w],
                                    x8[:, :, row * 128 : (row + 1) * 128],
                                    x8[:, :, slo : slo + w],
                                    start=st,
                                    stop=last,
                                    perf_mode=DR,
                                    skip_group_check=True,
                                )
                        # ---- q^T chunks with fused gelu ----
                        gq = gqpool.tile([128, CCH, 512], BF16, tag="gq")
                        gq_ts.append(gq)
                        for jch in range(CCH):
                            pq = ps_pq.tile([128, 512], F32, tag="pq")
                            for cch in range(CCH):
                                nc.tensor.matmul(
                                    pq[:],
                                    w_qkv[:, cch, jch * 128 : (jch + 1) * 128],
                                    xT[:, cch, :],
                                    start=(cch == 0),
                                    stop=(cch == CCH - 1),
                                )
                            nc.scalar.activation(gq[:, jch, :], pq[:], GELU)

                    # ===== pass 1b: G rows 3-5 from retained hi tiles =======
                    gB1 = ps_pq.tile([128, 512], F32, tag="pq", name="gB1")
                    gB2 = ps_pq.tile([128, 256], F32, tag="pq", name="gB2")
                    g_b = (gB1, gB2)
                    mirrors = list(MIRRORS_EARLY)[::-1]

                    def mirror_one(i, j, use_act=False):
                        pt = ps_tr.tile([128, 128], BF16, tag="tr", name="pt")
                        nc.tensor.transpose(
                            pt[:], G_sb[:, j, i * 128 : i * 128 + 128], ident[:]
                        )
                        if use_act:
                            nc.scalar.activation(
                                G_sb[:, i, j * 128 : j * 128 + 128], pt[:], COPY
                            )
                        else:
                            nc.vector.tensor_copy(
                                G_sb[:, i, j * 128 : j * 128 + 128], pt[:]
                            )

                    for i3 in range(3):  # G rows 0-2 evac (DVE/ACT mix)
                        w = C - i3 * 128
                        plo = (0, 768, 1408)[i3]
                        if i3 == 1:
                            nc.scalar.activation(
                                G_sb[:, i3, i3 * 128 : C], g_acc[:, plo : plo + w], COPY
                            )
                        else:
                            nc.vector.tensor_copy(
                                G_sb[:, i3, i3 * 128 : C], g_acc[:, plo : plo + w]
                            )
                    for tci in range(32):
                        x_hi = x_hi_keep[tci]
                        first = tci == 0
                        last = tci == 31
                        for row, tidx, plo, slo, w in G_B_SPLITS:
                            nc.tensor.matmul(
                                g_b[tidx][:, plo : plo + w],
                                x_hi[:, row * 128 - XLO : (row + 1) * 128 - XLO],
                                x_hi[:, slo : slo + w],
                                start=(first and plo == 0),
                                stop=last,
                                skip_group_check=True,
                            )
                        if tci % 4 == 0 and defer_pb:
                            dst, srcap = defer_pb.pop()
                            nc.gpsimd.dma_start(dst, srcap)
                        if tci == 26 and b + 1 < BPC:
                            x_pre = load_x(b + 1, 0)
                        if tci == 29 and b + 1 < BPC:
                            x_pre2 = load_x(b + 1, 1)
                        if tci >= 3 and tci % 2 == 1 and mirrors:
                            mirror_one(*mirrors.pop(), use_act=(tci % 4 == 1))
                    while mirrors:
                        mirror_one(*mirrors.pop())
                    # rows 3-5 evac + remaining mirrors
                    nc.vector.tensor_copy(G_sb[:, 3, 384:768], gB1[:, 0:384])
                    nc.scalar.activation(G_sb[:, 4, 512:768], gB2[:], COPY)
                    nc.vector.tensor_copy(G_sb[:, 5, 640:768], gB1[:, 384:512])
                    for n, (i, j) in enumerate(MIRRORS_LATE):
                        mirror_one(i, j, use_act=(n % 2 == 1))

                if DEBUG_DUMPS and b == 0:
                    nc.sync.dma_start(dbg["G"][:], G_sb[:])
                    nc.sync.dma_start(dbg["gq"][:], gq_ts[0][:])

                # ===== chain: A = G @ W_k; kv^T = W_v^T A; W' ===============
                nc.gpsimd.memset(gkvT[:], 0.0)
                A_sb = apool.tile([128, CCH, C], BF16, tag="A")
                W_sb = apool.tile([128, NPAIR, C], BF16, tag="Wp")
                with tc.tile_pool(name="ps_post", bufs=2, space=PSUM) as ps_post:
                    for cp in range(CCH):
                        pA = ps_post.tile([128, C], F32, tag="post")
                        for lo, hi in ((0, 512), (512, 768)):
                            for cch in range(CCH):
                                nc.tensor.matmul(
                                    pA[:, lo:hi],
                                    G_sb[:, cch, cp * 128 : (cp + 1) * 128],
                                    w_qkv[:, cch, C + lo : C + hi],
                                    start=(cch == 0),
                                    stop=(cch == CCH - 1),
                                    skip_group_check=True,
                                )
                        nc.vector.tensor_copy(A_sb[:, cp, 0:384], pA[:, 0:384])
                        nc.scalar.activation(A_sb[:, cp, 384:768], pA[:, 384:768], COPY)

                    kv_acc = ps_post.tile([128, NPAIR * 128], F32, tag="post")
                    for pr in range(NPAIR):
                        psl = slice(pr * 128, pr * 128 + 128)
                        for cch in range(CCH):
                            # start clears the whole bank: first MM per bank only
                            nc.tensor.matmul(
                                kv_acc[:, psl],
                                w_qkv[:, cch, 2 * C + pr * 128 : 2 * C + (pr + 1) * 128],
                                A_sb[:, cch, pr * 128 : (pr + 1) * 128],
                                start=(cch == 0 and pr in (0, 4)),
                                stop=(cch == CCH - 1),
                                skip_group_check=True,
                            )
                    # gelu(kv^T * scale) into block-diagonal pair tiles (two
                    # batched ACTs: even-head halves, odd-head halves), then
                    # W'_pair = gkvT^T @ w_proj rows.
                    kv_v = kv_acc[:].rearrange("p (n f) -> p n f", n=NPAIR)
                    nc.scalar.activation(
                        gkvT[0:64, :, 0:64], kv_v[0:64, :, 0:64], GELU, scale=SCALE
                    )
                    nc.scalar.activation(
                        gkvT[64:128, :, 64:128], kv_v[64:128, :, 64:128], GELU,
                        scale=SCALE,
                    )
                    for pr in range(NPAIR):
                        pW = ps_post.tile([128, C], F32, tag="post", name="pW")
                        for lo, hi in ((0, 512), (512, 768)):
                            # each split is the first MM into its own bank
                            nc.tensor.matmul(
                                pW[:, lo:hi],
                                gkvT[:, pr, :],
                                w_proj[:, pr, lo:hi],
                                start=True,
                                stop=True,
                                skip_group_check=True,
                            )
                        nc.vector.tensor_copy(W_sb[:, pr, 0:384], pW[:, 0:384])
                        nc.scalar.activation(W_sb[:, pr, 384:768], pW[:, 384:768], COPY)

                    if DEBUG_DUMPS and b == 0:
                        nc.sync.dma_start(dbg["A"][:], A_sb[:])
                        nc.sync.dma_start(dbg["W"][:], W_sb[:])

                    # ================= pass 2: y = gq @ W' + b ==============
                    for ts in range(NTS):
                        gq = gq_ts[ts]
                        for tc4 in range(4):
                            tsl = slice(tc4 * 128, tc4 * 128 + 128)
                            py = ps_post.tile([128, C], F32, tag="post", name="py")
                            for pr in range(NPAIR):
                                lastp = pr == NPAIR - 1
                                nc.tensor.matmul(
                                    py[:, 0:512],
                                    gq[:, pr, tsl],
                                    W_sb[:, pr, 0:512],
                                    start=(pr == 0),
                                    stop=lastp,
                                    skip_group_check=True,
                                )
                                nc.tensor.matmul(
                                    py[:, 512:768],
                                    gq[:, pr, tsl],
                                    W_sb[:, pr, 512:768],
                                    start=(pr == 0),
                                    stop=lastp,
                                    skip_group_check=True,
                                )
                            y_sb = ypool.tile([128, C], F32)
                            nc.vector.tensor_add(
                                y_sb[:, 0:512], py[:, 0:512], b_bc[:, 0:512]
                            )
                            nc.scalar.activation(y_sb[:, 512:768], py[:, 512:768], COPY)
                            nc.gpsimd.tensor_add(
                                y_sb[:, 512:768], y_sb[:, 512:768], b_bc[:, 512:768]
                            )
                            t0 = ts * 512 + tc4 * 128
                            nc.sync.dma_start(y_d[b, t0 : t0 + 128, :], y_sb[:])

    nc.compile()
    return nc


_cached_nc = None


def kernel(x, w_qkv, w_proj, b_proj):
    global _cached_nc
    if _cached_nc is None:
        _cached_nc = _build_program()
    nc = _cached_nc

    x = np.ascontiguousarray(x, dtype=np.float32)
    in_maps = [
        {
            "x": x[i * BPC : (i + 1) * BPC],
            "w_qkv": np.asarray(w_qkv, dtype=np.float32),
            "w_proj": np.asarray(w_proj, dtype=np.float32),
            "b_proj": np.asarray(b_proj, dtype=np.float32),
        }
        for i in range(NCORES)
    ]
    last_err = None
    for _attempt in range(3):
        try:
            res = run_bass_kernel_spmd(nc, in_maps, core_ids=list(range(NCORES)))
            out = np.concatenate(
                [res.results[i]["y"] for i in range(NCORES)], axis=0
            )
            return out.astype(np.float32)
        except Exception as e:  # transient NRT device errors recover on retry
            last_err = e
    raise last_err


# revision 58
# speedup vs baseline: 1.0443x; 1.0006x over previous
"""Trainium2 Bass kernel for AttentionSimple (linear/kernelized attention).

Computes, for x:[B,N,C], w_qkv:[C,3C], w_proj:[C,C], b_proj:[C]:
    qkv = x @ w_qkv -> split q,k,v per head (H=12, D=64)
    kv  = (k^T v) * D^-0.5          per (b, h)     [D, D]
    out = gelu(q) @ gelu(kv)        per (b, h)     [N, D]
    y   = out @ w_proj + b_proj

Sharding: data-parallel over batch B=16 across 8 NeuronCores (2 batches/core).
All matmuls run in bf16 with fp32 PSUM accumulation.

Algorithm per core (per batch b), using the Gram trick
kv^T = W_v^T (x^T x) W_k (G = x^T x symmetric) and folding the attention
into the projection: y = gelu(q) @ W' with W'_h = gelu(kv)_h @ w_proj_h:

  pass 1a (per 512-token slice): x loaded once as lo[0:384]/hi[384:768]
      bf16 tiles (hi retained for pass 1b); x^T via PE transposes; G rows
      0-2 (upper triangle) accumulated in one packed 4-bank PSUM region;
      q^T chunks (lhsT = W_q chunk, rhs = x^T) with gelu fused into the
      ACT evacuation.
  pass 1b: G rows 3-5 from the retained hi tiles (no re-DMA); G evacs and
      the 15 mirror transposes interleaved.
  chain:  A = G @ W_k; kv^T pairs = W_v^T A; gelu(kv^T * scale) into
      block-diagonal pair tiles; W'_pair = gkvT^T @ w_proj rows.
  pass 2: y[tokens, C] = sum_pr gqT_pr^T @ W'_pair + bias; contiguous DMA.

Self-contained: hardcodes shapes; builds the Bass program, runs it SPMD on
cores 0-7 via bass_utils.run_bass_kernel_spmd, returns the gathered output.
"""

import numpy as np

import concourse.bacc as bacc
import concourse.bass as bass
import concourse.mybir as mybir
import concourse.tile as tile
from concourse import masks
from concourse.bass_utils import run_bass_kernel_spmd

F32 = mybir.dt.float32
BF16 = mybir.dt.bfloat16
FP8 = mybir.dt.float8e4
DR = mybir.MatmulPerfMode.DoubleRow
GELU = mybir.ActivationFunctionType.Gelu
COPY = mybir.ActivationFunctionType.Copy
PSUM = bass.MemorySpace.PSUM

B, N, C = 16, 4096, 768
H, D = 12, 64
SCALE = D**-0.5
NCORES = 8
BPC = B // NCORES  # batches per core
CCH = C // 128  # 6 column chunks of 128
NTS = N // 512  # 8 slices of 512 tokens
NPAIR = H // 2  # 6 head pairs (128 cols each)
XLO = 384  # x cols [0:384) in recycled lo tiles, [384:768) retained hi tiles

# G rows 0-2 packed into one [128, 1920] PSUM region (banks of 512 f32).
# (row, psum_lo, src_lo, width); computed with fp8 e4m3 DoubleRow matmuls
# (contraction = 256 tokens/pair-chunk); no MM crosses a 512-col PSUM bank.
G_A_SPLITS = [
    (0, 0, 0, 512),
    (0, 512, 512, 256),
    (1, 768, 128, 256),
    (1, 1024, 384, 384),
    (2, 1408, 256, 128),
    (2, 1536, 384, 384),
]
# G rows 3-5: tile1 holds row3 @[0:384) + row5 @[384:512), tile2 row4 @[0:256)
G_B_SPLITS = [
    (3, 0, 0, 0, 384),  # (row, tile_idx, psum_lo, src_lo(in hi), width)
    (4, 1, 0, 512 - XLO, 256),
    (5, 0, 384, 640 - XLO, 128),
]
MIRRORS_EARLY = [(i, j) for i in range(1, CCH) for j in range(min(i, 3))]
MIRRORS_LATE = [(4, 3), (5, 3), (5, 4)]


DEBUG_DUMPS = False


def _build_program():
    nc = bacc.Bacc("TRN2", target_bir_lowering=False, debug=False)

    dbg = {}
    if DEBUG_DUMPS:
        dbg["G"] = nc.dram_tensor("G_dbg", [128, CCH, C], BF16, kind="ExternalOutput").ap()
        dbg["gq"] = nc.dram_tensor("gq_dbg", [128, CCH, 512], BF16, kind="ExternalOutput").ap()
        dbg["A"] = nc.dram_tensor("A_dbg", [128, CCH, C], BF16, kind="ExternalOutput").ap()
        dbg["W"] = nc.dram_tensor("W_dbg", [128, NPAIR, C], BF16, kind="ExternalOutput").ap()

    x_d = nc.dram_tensor("x", [BPC, N, C], F32, kind="ExternalInput").ap()
    wq_d = nc.dram_tensor("w_qkv", [C, 3 * C], F32, kind="ExternalInput").ap()
    wp_d = nc.dram_tensor("w_proj", [C, C], F32, kind="ExternalInput").ap()
    bp_d = nc.dram_tensor("b_proj", [C], F32, kind="ExternalInput").ap()
    y_d = nc.dram_tensor("y", [BPC, N, C], F32, kind="ExternalOutput").ap()

    with tile.TileContext(nc) as tc:
        with (
            tc.tile_pool(name="weights", bufs=1) as wpool,
            tc.tile_pool(name="acts", bufs=1) as apool,
            tc.tile_pool(name="gq", bufs=8) as gqpool,
            tc.tile_pool(name="xlo", bufs=12) as xpool,
            tc.tile_pool(name="xhi", bufs=40) as xhipool,
            tc.tile_pool(name="xt", bufs=3) as xtpool,
            tc.tile_pool(name="x8", bufs=6) as x8pool,
            tc.tile_pool(name="yout", bufs=3) as ypool,
            tc.tile_pool(name="ps_pq", bufs=2, space=PSUM) as ps_pq,
        ):
            # ---- HAM warmup: dense dummy matmuls so the PE clock-gate
            # flips to 8/8 ~3.6us in instead of ~15us.
            scratch = wpool.tile([128, 128], BF16)
            nc.gpsimd.memset(scratch[:], 0.0)
            warm = ps_pq.tile([128, 512], F32, tag="pq", name="warm")
            for _ in range(32):
                nc.tensor.matmul(warm[:, 0:128], scratch[:], scratch[:], start=True,
                                 stop=True, skip_group_check=True)
            ident = wpool.tile([128, 128], BF16)
            masks.make_identity(nc, ident[:])

            # ---- x prefetch helpers (lo recycled, hi retained per batch;
            # fp8 e4m3 pair-interleaved copies for the DoubleRow G matmuls,
            # cast in the DMA so they exactly match e4m3(f32)) --
            def load_x(b, ts):
                tiles = []
                for tc4 in range(4):
                    t0 = ts * 512 + tc4 * 128
                    x_lo = xpool.tile([128, XLO], BF16, tag="x_lo")
                    nc.gpsimd.dma_start(x_lo[:], x_d[b, t0 : t0 + 128, 0:XLO])
                    x_hi = xhipool.tile([128, C - XLO], BF16, tag="x_hi")
                    nc.gpsimd.dma_start(x_hi[:], x_d[b, t0 : t0 + 128, XLO:C])
                    tiles.append((x_lo, x_hi))
                return tiles

            # ---- weights: q slices interleaved with the first x
            # prefetches on the gpsimd ring so neither starves the other.
            w_qkv = wpool.tile([128, CCH, 3 * C], BF16)
            w_proj = wpool.tile([128, CCH, C], BF16)
            x_pre = load_x(0, 0)
            qw = [
                (w_qkv[:, cch, lo:hi], wq_d[cch * 128 : (cch + 1) * 128, lo:hi])
                for lo, hi in ((0, 512), (512, 768))
                for cch in range(CCH)
            ]
            for dst, srcap in qw[:6]:
                nc.gpsimd.dma_start(dst, srcap)
            x_pre2 = load_x(0, 1)
            for dst, srcap in qw[6:]:
                nc.gpsimd.dma_start(dst, srcap)
            b_bc = wpool.tile([128, C], F32)
            # k+v weight loads dribble through pass 1a (2/ts); proj+bias and
            # the next batch's first x slices move to pass 1b, where the
            # gpsimd ring is otherwise idle.
            defer_kv = []
            for cch in range(CCH):  # k part (A-stage consumes first)
                defer_kv.append(
                    (w_qkv[:, cch, C : 2 * C], wq_d[cch * 128 : (cch + 1) * 128, C : 2 * C])
                )
            for cch in range(CCH):  # v part (kv-stage)
                defer_kv.append(
                    (w_qkv[:, cch, 2 * C :], wq_d[cch * 128 : (cch + 1) * 128, 2 * C :])
                )
            defer_kv.reverse()
            defer_pb = [(b_bc[:], bp_d.unsqueeze(0).partition_broadcast(128))]
            for cch in range(CCH):
                defer_pb.append(
                    (w_proj[:, cch, :], wp_d[cch * 128 : (cch + 1) * 128, :])
                )
            defer_pb.reverse()

            gkvT = apool.tile([128, NPAIR, 128], BF16, tag="gkv")

            for b in range(BPC):
                # gqT: gelu(q)^T, [c=768, t=4096] as 6 chunks, per-ts tiles
                gq_ts = []
                # G (bf16, both triangles after mirrors)
                G_sb = apool.tile([128, CCH, C], BF16, tag="G")
                x_hi_keep = []  # [32][tile] retained hi tiles for pass 1b

                # ===== pass 1a: x^T, G rows 0-2, gelu(q)^T ==================
                # ps_tr is batch-scoped: it frees its 2 banks after pass 1b
                # so the chain/pass-2 pool below can run 3 slots deep.
                with (
                    tc.tile_pool(name="ps_gA", bufs=1, space=PSUM) as ps_gA,
                    tc.tile_pool(name="ps_tr", bufs=2, space=PSUM) as ps_tr,
                ):
                    g_acc = ps_gA.tile([128, 1920], F32, name="gA")
                    for ts in range(NTS):
                        xT = xtpool.tile([128, CCH, 512], BF16)
                        x_tiles = x_pre
                        x_pre = x_pre2
                        if ts + 2 < NTS:
                            x_pre2 = load_x(b, ts + 2)
                        for _ in range(2):
                            if defer_kv:
                                dst, srcap = defer_kv.pop()
                                nc.gpsimd.dma_start(dst, srcap)
                        for tc4 in range(4):
                            x_lo, x_hi = x_tiles[tc4]
                            x_hi_keep.append(x_hi)
                            # PE transposes -> xT (batched DVE evacuation)
                            tr = ps_tr.tile([128, CCH * 128], BF16, tag="tr")
                            for cch in range(CCH):
                                src = (
                                    x_lo[:, cch * 128 : (cch + 1) * 128]
                                    if cch < 3
                                    else x_hi[:, cch * 128 - XLO : (cch + 1) * 128 - XLO]
                                )
                                nc.tensor.transpose(
                                    tr[:, cch * 128 : (cch + 1) * 128], src, ident[:]
                                )
                            nc.vector.tensor_copy(
                                xT[:, :, tc4 * 128 : tc4 * 128 + 128],
                                tr[:].rearrange("p (c f) -> p c f", c=CCH),
                            )
                            # fp8 e4m3 copies for the DoubleRow G matmuls
                            if tc4 % 2 == 0:
                                x8 = x8pool.tile([128, 2, C], FP8, tag="x8")
                            nc.vector.tensor_copy(x8[:, tc4 % 2, 0:XLO], x_lo[:])
                            nc.vector.tensor_copy(x8[:, tc4 % 2, XLO:C], x_hi[:])
                            if tc4 % 2 == 0:
                                continue
                            # G rows 0-2: fp8 DoubleRow, 256-token contraction
                            first = ts == 0 and tc4 == 1
                            last = ts == NTS - 1 and tc4 == 3
                            seen_banks = set()
                            for row, plo, slo, w in G_A_SPLITS:
                                bank = plo // 512
                                st = first and bank not in seen_banks
                                seen_banks.add(bank)
                                nc.tensor.matmul(
                                    g_acc[:, plo : plo + w],
                                    x8[:, :, row * 128 : (row + 1) * 128],
                                    x8[:, :, slo : slo + w],
                                    start=st,
                                    stop=last,
                                    perf_mode=DR,
                                    skip_group_check=True,
                                )
                        # ---- q^T chunks with fused gelu ----
                        gq = gqpool.tile([128, CCH, 512], BF16, tag="gq")
                        gq_ts.append(gq)
                        for jch in range(CCH):
                            pq = ps_pq.tile([128, 512], F32, tag="pq")
                            for cch in range(CCH):
                                nc.tensor.matmul(
                                    pq[:],
                                    w_qkv[:, cch, jch * 128 : (jch + 1) * 128],
                                    xT[:, cch, :],
                                    start=(cch == 0),
                                    stop=(cch == CCH - 1),
                                )
                            nc.scalar.activation(gq[:, jch, :], pq[:], GELU)

                    # ===== pass 1b: G rows 3-5 from retained hi tiles =======
                    gB1 = ps_pq.tile([128, 512], F32, tag="pq", name="gB1")
                    gB2 = ps_pq.tile([128, 256], F32, tag="pq", name="gB2")
                    g_b = (gB1, gB2)
                    mirrors = list(MIRRORS_EARLY)[::-1]

                    def mirror_one(i, j, use_act=False):
                        pt = ps_tr.tile([128, 128], BF16, tag="tr", name="pt")
                        nc.tensor.transpose(
                            pt[:], G_sb[:, j, i * 128 : i * 128 + 128], ident[:]
                        )
                        if use_act:
                            nc.scalar.activation(
                                G_sb[:, i, j * 128 : j * 128 + 128], pt[:], COPY
                            )
                        else:
                            nc.vector.tensor_copy(
                                G_sb[:, i, j * 128 : j * 128 + 128], pt[:]
                            )

                    for i3 in range(3):  # G rows 0-2 evac (DVE/ACT mix)
                        w = C - i3 * 128
                        plo = (0, 768, 1408)[i3]
                        if i3 == 1:
                            nc.scalar.activation(
                                G_sb[:, i3, i3 * 128 : C], g_acc[:, plo : plo + w], COPY
                            )
                        else:
                            nc.vector.tensor_copy(
                                G_sb[:, i3, i3 * 128 : C], g_acc[:, plo : plo + w]
                            )
                    for tci in range(32):
                        x_hi = x_hi_keep[tci]
                        first = tci == 0
                        last = tci == 31
                        for row, tidx, plo, slo, w in G_B_SPLITS:
                            nc.tensor.matmul(
                                g_b[tidx][:, plo : plo + w],
                                x_hi[:, row * 128 - XLO : (row + 1) * 128 - XLO],
                                x_hi[:, slo : slo + w],
                                start=(first and plo == 0),
                                stop=last,
                                skip_group_check=True,
                            )
                        if tci % 4 == 0 and defer_pb:
                            dst, srcap = defer_pb.pop()
                            nc.gpsimd.dma_start(dst, srcap)
                        if tci == 26 and b + 1 < BPC:
                            x_pre = load_x(b + 1, 0)
                        if tci == 29 and b + 1 < BPC:
                            x_pre2 = load_x(b + 1, 1)
                        if tci >= 3 and tci % 2 == 1 and mirrors:
                            mirror_one(*mirrors.pop(), use_act=(tci % 4 == 1))
                    while mirrors:
                        mirror_one(*mirrors.pop())
                    # rows 3-5 evac + remaining mirrors
                    nc.vector.tensor_copy(G_sb[:, 3, 384:768], gB1[:, 0:384])
                    nc.scalar.activation(G_sb[:, 4, 512:768], gB2[:], COPY)
                    nc.vector.tensor_copy(G_sb[:, 5, 640:768], gB1[:, 384:512])
                    for n, (i, j) in enumerate(MIRRORS_LATE):
                        mirror_one(i, j, use_act=(n % 2 == 1))

                if DEBUG_DUMPS and b == 0:
                    nc.sync.dma_start(dbg["G"][:], G_sb[:])
                    nc.sync.dma_start(dbg["gq"][:], gq_ts[0][:])

                # ===== chain: A = G @ W_k; kv^T = W_v^T A; W' ===============
                nc.gpsimd.memset(gkvT[:], 0.0)
                A_sb = apool.tile([128, CCH, C], BF16, tag="A")
                W_sb = apool.tile([128, NPAIR, C], BF16, tag="Wp")
                with tc.tile_pool(name="ps_post", bufs=3, space=PSUM) as ps_post:
                    for cp in range(CCH):
                        pA = ps_post.tile([128, C], F32, tag="post")
                        for lo, hi in ((0, 512), (512, 768)):
                            for cch in range(CCH):
                                nc.tensor.matmul(
                                    pA[:, lo:hi],
                                    G_sb[:, cch, cp * 128 : (cp + 1) * 128],
                                    w_qkv[:, cch, C + lo : C + hi],
                                    start=(cch == 0),
                                    stop=(cch == CCH - 1),
                                    skip_group_check=True,
                                )
                        nc.vector.tensor_copy(A_sb[:, cp, 0:384], pA[:, 0:384])
                        nc.scalar.activation(A_sb[:, cp, 384:768], pA[:, 384:768], COPY)

                    kv_acc = ps_post.tile([128, NPAIR * 128], F32, tag="post")
                    for pr in range(NPAIR):
                        psl = slice(pr * 128, pr * 128 + 128)
                        for cch in range(CCH):
                            # start clears the whole bank: first MM per bank only
                            nc.tensor.matmul(
                                kv_acc[:, psl],
                                w_qkv[:, cch, 2 * C + pr * 128 : 2 * C + (pr + 1) * 128],
                                A_sb[:, cch, pr * 128 : (pr + 1) * 128],
                                start=(cch == 0 and pr in (0, 4)),
                                stop=(cch == CCH - 1),
                                skip_group_check=True,
                            )
                    # gelu(kv^T * scale) into block-diagonal pair tiles (two
                    # batched ACTs: even-head halves, odd-head halves), then
                    # W'_pair = gkvT^T @ w_proj rows.
                    kv_v = kv_acc[:].rearrange("p (n f) -> p n f", n=NPAIR)
                    nc.scalar.activation(
                        gkvT[0:64, :, 0:64], kv_v[0:64, :, 0:64], GELU, scale=SCALE
                    )
                    nc.scalar.activation(
                        gkvT[64:128, :, 64:128], kv_v[64:128, :, 64:128], GELU,
                        scale=SCALE,
                    )
                    for pr in range(NPAIR):
                        pW = ps_post.tile([128, C], F32, tag="post", name="pW")
                        for lo, hi in ((0, 512), (512, 768)):
                            # each split is the first MM into its own bank
                            nc.tensor.matmul(
                                pW[:, lo:hi],
                                gkvT[:, pr, :],
                                w_proj[:, pr, lo:hi],
                                start=True,
                                stop=True,
                                skip_group_check=True,
                            )
                        nc.vector.tensor_copy(W_sb[:, pr, 0:384], pW[:, 0:384])
                        nc.scalar.activation(W_sb[:, pr, 384:768], pW[:, 384:768], COPY)

                    if DEBUG_DUMPS and b == 0:
                        nc.sync.dma_start(dbg["A"][:], A_sb[:])
                        nc.sync.dma_start(dbg["W"][:], W_sb[:])

                    # ================= pass 2: y = gq @ W' + b ==============
                    for ts in range(NTS):
                        gq = gq_ts[ts]
                        for tc4 in range(4):
                            tsl = slice(tc4 * 128, tc4 * 128 + 128)
                            py = ps_post.tile([128, C], F32, tag="post", name="py")
                            for pr in range(NPAIR):
                                lastp = pr == NPAIR - 1
                                nc.tensor.matmul(
                                    py[:, 0:512],
                                    gq[:, pr, tsl],
                                    W_sb[:, pr, 0:512],
                                    start=(pr == 0),
                                    stop=lastp,
                                    skip_group_check=True,
                                )
                                nc.tensor.matmul(
                                    py[:, 512:768],
                                    gq[:, pr, tsl],
                                    W_sb[:, pr, 512:768],
                                    start=(pr == 0),
                                    stop=lastp,
                                    skip_group_check=True,
                                )
                            y_sb = ypool.tile([128, C], F32)
                            nc.vector.tensor_add(
                                y_sb[:, 0:512], py[:, 0:512], b_bc[:, 0:512]
                            )
                            nc.scalar.activation(y_sb[:, 512:768], py[:, 512:768], COPY)
                            nc.gpsimd.tensor_add(
                                y_sb[:, 512:768], y_sb[:, 512:768], b_bc[:, 512:768]
                            )
                            t0 = ts * 512 + tc4 * 128
                            nc.sync.dma_start(y_d[b, t0 : t0 + 128, :], y_sb[:])

    nc.compile()
    return nc


_cached_nc = None


def kernel(x, w_qkv, w_proj, b_proj):
    global _cached_nc
    if _cached_nc is None:
        _cached_nc = _build_program()
    nc = _cached_nc

    x = np.ascontiguousarray(x, dtype=np.float32)
    in_maps = [
        {
            "x": x[i * BPC : (i + 1) * BPC],
            "w_qkv": np.asarray(w_qkv, dtype=np.float32),
            "w_proj": np.asarray(w_proj, dtype=np.float32),
            "b_proj": np.asarray(b_proj, dtype=np.float32),
        }
        for i in range(NCORES)
    ]
    last_err = None
    for _attempt in range(3):
        try:
            res = run_bass_kernel_spmd(nc, in_maps, core_ids=list(range(NCORES)))
            out = np.concatenate(
                [res.results[i]["y"] for i in range(NCORES)], axis=0
            )
            return out.astype(np.float32)
        except Exception as e:  # transient NRT device errors recover on retry
            last_err = e
    raise last_err
